# revision 11
# baseline (speedup 1.0000x reference)
"""Trainium2 Bass kernel for the fused einsum/groupconv/bmm module (v2).

Math (per image n, C=256, H=W=56, HW=3136):
  t1[c,e] = sum_s X[c,s] P[s,e]
  t3      = groupconv3x3(x[n], conv_w, groups=2)
  t4      = p4 * t1;  t5[a] = sum_b t4[a,b] p5[b]
  t6      = (t4 @ t3) / 16;  t7[s] = (sum_c t5[c] X[c,s]) / 16
  out     = t6 + t7[broadcast over c]

v2 design (8 cores, 4 images each, bf16 datapath, fp32 PSUM accumulate):
  - All HBM I/O in bf16 with contiguous per-partition layouts (host does
    every relayout): big DMA descriptors only, ~2x fewer wire bytes.
  - X^T built by XBAR dma_start_transpose (2 per image) instead of PE
    transposes + PSUM copies; contiguous dests only (strided XBAR dests
    are wrong on HW).
  - Padded conv layout built on-chip: pad cells memset once per physical
    buffer, interior refreshed per image by one strided copy per c-block.
  - t7 folded into the t6 PSUM accumulation as two rank-128 matmuls with
    a broadcast-t5 lhsT; 1/sqrt(C) folded into p4 host-side.
  - Outputs accumulated into a whole-image SBUF tile, one DMA per
    (image, channel-half); the last image's store is split so the kernel
    tail doesn't serialize behind the full image.
"""

import sys

sys.path.insert(0, "/opt/trn_rl_repo")

import numpy as np

N, C, H, W = 32, 256, 56, 56
HW = H * W            # 3136
PH = H + 2            # 58
PHW = PH * PH         # 3364
XLEN = PHW + 2        # +2 tail guard for the last chunk's corner taps
NCORES = 8
NPER = N // NCORES    # 4 images per core
CHP = 8 * PH          # conv chunk: 8 padded rows = 464 (fits a PSUM bank)
CHI = 8 * W           # interior chunk = 448
NCHUNK = 7
SPAD = 3200           # s padded to 25*128 for XBAR/t1 chunking
KT = SPAD // 128      # 25
INV = 1.0 / 16.0      # 1/sqrt(C)


def build_body(tc, outs, ins):
    import concourse.bass as bass
    import concourse.mybir as mybir

    nc = tc.nc
    f32 = mybir.dt.float32
    bf16 = mybir.dt.bfloat16

    # DRAM scratch ring for the t7 partition-broadcast round-trip
    t7sc_d = nc.dram_tensor("t7sc", (8, CHI), bf16, kind="Internal").ap()

    x_d = ins["x"]          # (NPER, C, HW)        bf16
    p1_d = ins["p1"]        # (128, KT, C)         bf16  [p,k,e] = p1[k*128+p, e]
    wt_d = ins["wt"]        # (128, 2, 9, 128)     bf16  [i,g,t,o]
    p4_d = ins["p4"]        # (128, 2, C)          f32   [a_lo,ab,b] = p4[ab*128+a_lo, b]/16
    p5_d = ins["p5"]        # (128, 2)             bf16  [b,bb] = p5[bb*128+b]
    out_d = outs["out"]     # (NPER, 2, 128, HW)   bf16

    with (
        tc.tile_pool(name="const", bufs=1) as constp,
        tc.tile_pool(name="xtp", bufs=2) as xtp,
        tc.tile_pool(name="svp", bufs=2) as svp,
        tc.tile_pool(name="t3p", bufs=16) as t3p,
        tc.tile_pool(name="outp", bufs=2) as outp,
        tc.tile_pool(name="ps_acc", bufs=2, space="PSUM") as ps_acc,
        tc.tile_pool(name="ps_cv", bufs=2, space="PSUM") as ps_cv,
        tc.tile_pool(name="ps_t6", bufs=2, space="PSUM") as ps_t6,
    ):
        # ---- constants (loaded on the Activation DMA queue so image 0's
        # x loads own the SP queue from t=0; wt first since conv needs it) ----
        p1_sb = constp.tile([128, KT * C], bf16, name="p1_sb")
        wt_sb = constp.tile([128, 2 * 9 * 128], bf16, name="wt_sb")
        p4_sb = constp.tile([128, 2 * C], f32, name="p4_sb")
        p5_sb = constp.tile([128, 2], bf16, name="p5_sb")
        wt_flat = wt_d.rearrange("i g t o -> i (g t o)")
        nc.scalar.dma_start(out=wt_sb[:, 0:1152], in_=wt_flat[:, 0:1152])
        nc.scalar.dma_start(out=wt_sb[:, 1152:2304], in_=wt_flat[:, 1152:2304])
        nc.scalar.dma_start(
            out=p4_sb[:, :], in_=p4_d.rearrange("b bb a -> b (bb a)")
        )
        nc.scalar.dma_start(out=p5_sb[:, :], in_=p5_d)

        # ---- persistent x buffers (explicit A/B double buffering) ----
        # xunp: contiguous x + 64 zero tail cols (zeroed once per buffer).
        # xpad: 58x58 zero-padded layout (+2 guard); pads memset once per
        # buffer, interior rewritten per image.
        xunp = [[None, None], [None, None]]
        xpad = [[None, None], [None, None]]
        for buf in range(2):
            for cb in range(2):
                xu = constp.tile([128, SPAD], bf16, name=f"xu{buf}{cb}")
                nc.vector.memset(xu[:, HW:SPAD], 0.0)
                xunp[buf][cb] = xu
                xp = constp.tile([128, XLEN], bf16, name=f"xq{buf}{cb}")
                # zero only the pad cells (head guard + row 0, the 56 row
                # seams, row 57 + tail guard); interior is rewritten per image
                nc.gpsimd.memset(xp[:, 0:59], 0.0)
                nc.gpsimd.memset(
                    xp[:, 58 : 58 + 57 * PH]
                    .rearrange("p (r w) -> p r w", w=PH)[:, :, 0:2],
                    0.0,
                )
                nc.gpsimd.memset(xp[:, 1 + 57 * PH : XLEN], 0.0)
                xpad[buf][cb] = xp

        def pad_copy(eng, buf, cb, r0, r1):
            """Copy x rows [r0,r1) into the padded interior (offset-1 flat
            layout: head guard cell keeps conv tap offsets >= 0)."""
            copy_fn = eng.copy if eng is nc.scalar else eng.tensor_copy
            copy_fn(
                xpad[buf][cb][:, 1 : 1 + PHW]
                .rearrange("p (r w) -> p r w", w=PH)[:, 1 + r0 : 1 + r1, 1:57],
                xunp[buf][cb][:, r0 * W : r1 * W]
                .rearrange("p (r w) -> p r w", w=W),
            )

        def load_image(n):
            """DMA x[n] in, build X^T chunks and padded layout."""
            buf = n % 2
            if n == 0:
                # cold start: land the first 10 rows of each c-block early so
                # conv chunk 0 can start ~1.3us in instead of ~5us
                for cb in range(2):
                    nc.sync.dma_start(
                        out=xunp[buf][cb][:, 0 : 10 * W],
                        in_=x_d[n, cb * 128 : (cb + 1) * 128, 0 : 10 * W],
                    )
                pad_copy(nc.vector, buf, 0, 0, 10)
                pad_copy(nc.scalar, buf, 1, 0, 10)
                for cb in range(2):
                    nc.sync.dma_start(
                        out=xunp[buf][cb][:, 10 * W : HW],
                        in_=x_d[n, cb * 128 : (cb + 1) * 128, 10 * W : HW],
                    )
                pad_copy(nc.vector, buf, 0, 10, 56)
                pad_copy(nc.scalar, buf, 1, 10, 56)
            else:
                for cb in range(2):
                    nc.sync.dma_start(
                        out=xunp[buf][cb][:, 0:HW],
                        in_=x_d[n, cb * 128 : (cb + 1) * 128, :],
                    )
                pad_copy(nc.vector, buf, 0, 0, 56)
                pad_copy(nc.gpsimd, buf, 1, 0, 56)
            xts = []
            for cb in range(2):
                xt = xtp.tile([128, KT * 128], bf16, tag=f"xt{cb}",
                              name=f"xt{cb}_{n}")
                nc.sync.dma_start_transpose(
                    xt.rearrange("p (k e) -> p k e", e=128),
                    xunp[buf][cb][:, :],
                )
                xts.append(xt)
            return xts

        xts_cur = load_image(0)
        # p1 rides the SP queue behind image 0's loads/XBARs; t1 needs it
        # only after image 0's conv block (~29us in)
        nc.sync.dma_start(
            out=p1_sb[:, :], in_=p1_d.rearrange("p k e -> p (k e)")
        )

        def t14(n, xts):
            """t1 (untransposed) -> t4 -> XBAR-transposed t4T blocks.

            t1'[c,e] = sum_s x[c,s] p1[s,e] via lhsT = X^T chunks; then
            t4' = p4/16 * t1' elementwise, and t4T[ab][b_lo, bb, a_lo] =
            t4'[ab][a_lo, bb*128+b_lo] via one XBAR transpose per a-block.
            """
            t4Ts = []
            t4ps = []
            for cb in range(2):
                t1ps = ps_acc.tile([128, C], f32, tag="t1", name=f"t1ps{cb}")
                for k in range(KT):
                    nc.tensor.matmul(
                        t1ps[:, :],
                        xts[cb][:, k * 128 : (k + 1) * 128],
                        p1_sb[:, k * C : (k + 1) * C],
                        start=(k == 0),
                        stop=(k == KT - 1),
                    )
                t4p = svp.tile([128, C], bf16, tag="t4p", name=f"t4p{cb}_{n}")
                nc.vector.tensor_mul(
                    t4p[:, :], t1ps[:, :], p4_sb[:, cb * C : (cb + 1) * C]
                )
                t4T = svp.tile([128, C], bf16, tag="t4T", bufs=4,
                               name=f"t4T{cb}_{n}")
                nc.sync.dma_start_transpose(
                    t4T.rearrange("p (kb a) -> p kb a", a=128), t4p[:, :]
                )
                t4Ts.append(t4T)
                t4ps.append(t4p)
            return t4Ts

        def t5part(n, t4Ts):
            """t5 column and broadcast-t5 lhsT (PE matmuls; the DVE
            tensor_tensor_reduce variant wedges real hardware)."""
            t5ps = ps_acc.tile([128, 2], f32, tag="t7p", bufs=2,
                               name=f"t5ps_{n}")
            for ab in range(2):
                for bb in range(2):
                    nc.tensor.matmul(
                        t5ps[:, ab : ab + 1],
                        t4Ts[ab][:, bb * 128 : (bb + 1) * 128],
                        p5_sb[:, bb : bb + 1],
                        start=(bb == 0),
                        stop=(bb == 1),
                    )
            t5col = svp.tile([128, 2], bf16, tag="t5c", name=f"t5col_{n}")
            nc.scalar.copy(t5col[:, :], t5ps[:, :])
            return t5col

        def t7make(n, c, t5col, buf):
            """t7 row for chunk c, replicated across partitions off the PE.

            Two M=1 matmuls build t7row [1,448] in PSUM; it round-trips
            through a DRAM scratch slot so a gpsimd (software-DGE) DMA with
            a stride-0 partition AP can replicate it to all 128 partitions —
            the add into t6 then rides the existing extraction copy on DVE
            instead of two rank-128 PE matmuls per (chunk, ab).
            """
            t7ps = ps_acc.tile([1, CHI], f32, tag="t7p", bufs=2,
                               name=f"t7ps_{n}_{c}")
            for cb in range(2):
                nc.tensor.matmul(
                    t7ps[:, :],
                    t5col[:, cb : cb + 1],
                    xunp[buf][cb][:, c * CHI : (c + 1) * CHI],
                    start=(cb == 0),
                    stop=(cb == 1),
                )
            t7row = svp.tile([1, CHI], bf16, tag="t7r", bufs=2,
                             name=f"t7row_{n}_{c}")
            nc.scalar.copy(t7row[:, :], t7ps[:, :])
            slot = (n * NCHUNK + c) % 8
            scr = t7sc_d[slot : slot + 1, :]
            nc.sync.dma_start(out=scr, in_=t7row[:, :])
            t7rep = svp.tile([128, CHI], bf16, tag="t7rep", bufs=6,
                             name=f"t7rep_{n}_{c}")
            nc.gpsimd.dma_start(
                out=t7rep[:, :],
                in_=bass.AP(
                    tensor=scr.tensor,
                    offset=scr.offset,
                    ap=[[0, 128]] + list(scr.ap[1:]),
                ),
            )
            return t7rep

        for n in range(NPER):
            buf = n % 2

            if n > 0:
                t4Ts = t14(n, xts_cur)

            # prefetch next image while PE runs this image's chunk loop
            xts_next = load_image(n + 1) if n + 1 < NPER else None

            # ---- chunk loop: conv(c) then t6(c-4), software-pipelined ----
            obig = outp.tile([128, 2 * HW], bf16, tag="ob", name=f"ob_{n}")
            t3cs = {}
            t7reps = {}

            def conv(c):
                r0 = 1 + 8 * c
                pair = []
                for g in range(2):
                    cv = ps_cv.tile([128, CHP], f32, tag="cv",
                                    name=f"cv{g}_{n}_{c}")
                    for tap in range(9):
                        kh, kw = tap // 3, tap % 3
                        foff = (r0 + kh - 1) * PH + kw
                        nc.tensor.matmul(
                            cv[:, :],
                            wt_sb[:, (g * 9 + tap) * 128 : (g * 9 + tap) * 128 + 128],
                            xpad[buf][g][:, foff : foff + CHP],
                            start=(tap == 0),
                            stop=(tap == 8),
                        )
                    t3g = t3p.tile([128, CHI], bf16, tag="t3",
                                   name=f"t3g{g}_{n}_{c}")
                    eng = nc.vector if g == 0 else nc.scalar
                    eng_copy = (eng.tensor_copy if g == 0 else eng.copy)
                    eng_copy(
                        t3g.rearrange("p (r w) -> p r w", w=W),
                        cv.rearrange("p (r w) -> p r w", w=PH)[:, :, 1:57],
                    )
                    pair.append(t3g)
                t3cs[c] = pair

            def t6(c):
                pair = t3cs.pop(c)
                t7rep = t7reps.pop(c)
                for ab in range(2):
                    t6ps = ps_t6.tile([128, CHI], f32, tag="t6",
                                      name=f"t6ps{ab}_{n}_{c}")
                    nc.tensor.matmul(
                        t6ps[:, :],
                        t4Ts[ab][:, 0:128],
                        pair[0][:, :],
                        start=True, stop=False,
                    )
                    nc.tensor.matmul(
                        t6ps[:, :],
                        t4Ts[ab][:, 128:256],
                        pair[1][:, :],
                        start=False, stop=True,
                    )
                    # t7 broadcast-add fused into the extraction (DVE only:
                    # Pool can't read PSUM, Act can't add two tensors)
                    nc.vector.tensor_add(
                        obig[:, ab * HW + c * CHI : ab * HW + (c + 1) * CHI],
                        t6ps[:, :],
                        t7rep[:, :],
                    )

            def flush(lo, hi):
                # flush finished output chunks early so the final store
                # doesn't serialize behind the whole image
                for ab in range(2):
                    nc.sync.dma_start(
                        out=out_d[n, ab][:, lo * CHI : hi * CHI],
                        in_=obig[:, ab * HW + lo * CHI : ab * HW + hi * CHI],
                    )

            # split the store only on the last image, where it shortens the
            # kernel tail; elsewhere one whole-buffer DMA per ab is cheaper
            last = n == NPER - 1

            if n == 0:
                # cold start: conv only needs the padded x (ready ~5us in),
                # while t1 waits on the XBAR transposes — run conv first
                for c in range(NCHUNK):
                    conv(c)
                t4Ts = t14(n, xts_cur)
                t5col = t5part(n, t4Ts)
                for c in range(3):
                    t7reps[c] = t7make(n, c, t5col, buf)
                for c in range(NCHUNK):
                    if c + 3 < NCHUNK:
                        t7reps[c + 3] = t7make(n, c + 3, t5col, buf)
                    t6(c)
                    if last and c == 4:
                        flush(0, 4)
                    elif last and c == 5:
                        flush(4, 6)
            else:
                next_t7 = 0
                for c in range(NCHUNK):
                    conv(c)
                    if c == 2:
                        # t5/t6 consume the XBAR'd t4T; two conv chunks
                        # (~7us) of slack in case real XBAR latency exceeds
                        # the cost model's 14ns/tile
                        t5col = t5part(n, t4Ts)
                    elif c >= 3:
                        for _ in range(2):
                            if next_t7 < NCHUNK:
                                t7reps[next_t7] = t7make(
                                    n, next_t7, t5col, buf
                                )
                                next_t7 += 1
                        if c >= 4:
                            t6(c - 4)
                if last:
                    flush(0, 3)
                for c in range(3, NCHUNK):
                    t6(c)
                    if last and c == 5:
                        flush(3, 6)

            if last:
                flush(6, NCHUNK)
            else:
                flush(0, NCHUNK)
            xts_cur = xts_next


_CACHE = {}


def _get_nc():
    if "nc" in _CACHE:
        return _CACHE["nc"]
    import concourse.bacc as bacc
    import concourse.mybir as mybir
    import concourse.tile as tile

    f32 = mybir.dt.float32
    bf16 = mybir.dt.bfloat16
    nc = bacc.Bacc(
        "TRN2", target_bir_lowering=False, debug=False, num_devices=NCORES
    )
    ins = {
        "x": nc.dram_tensor("x", (NPER, C, HW), bf16, kind="ExternalInput").ap(),
        "p1": nc.dram_tensor("p1", (128, KT, C), bf16, kind="ExternalInput").ap(),
        "wt": nc.dram_tensor("wt", (128, 2, 9, 128), bf16, kind="ExternalInput").ap(),
        "p4": nc.dram_tensor("p4", (128, 2, C), f32, kind="ExternalInput").ap(),
        "p5": nc.dram_tensor("p5", (128, 2), bf16, kind="ExternalInput").ap(),
    }
    outs = {
        "out": nc.dram_tensor(
            "out", (NPER, 2, 128, HW), bf16, kind="ExternalOutput"
        ).ap(),
    }
    with tile.TileContext(nc) as tc:
        build_body(tc, outs, ins)
    nc.compile()
    _CACHE["nc"] = nc
    return nc


def host_prep(inputs):
    """Split full inputs into per-core in_maps (host-side relayout + bf16)."""
    import ml_dtypes

    bf16 = ml_dtypes.bfloat16
    x = np.asarray(inputs["x"], dtype=np.float32).reshape(N, C, HW)
    p1 = np.asarray(inputs["p1_w"], dtype=np.float32)[..., 0].reshape(HW, C)
    p1p = np.zeros((SPAD, C), dtype=np.float32)
    p1p[0:HW] = p1
    p1h = np.ascontiguousarray(
        p1p.reshape(KT, 128, C).transpose(1, 0, 2)
    ).astype(bf16)
    wt = np.asarray(inputs["conv_w"], dtype=np.float32)  # (256, 128, 3, 3)
    wth = np.ascontiguousarray(
        wt.reshape(2, 128, 128, 9).transpose(2, 0, 3, 1)
    ).astype(bf16)  # [i, g, t, o]
    p4 = np.asarray(inputs["p4_w"], dtype=np.float32)[0]  # (a, b)
    p4h = np.ascontiguousarray(
        (p4 * INV).reshape(2, 128, C).transpose(1, 0, 2)
    )  # [a_lo, ab, b]  # [b, bb, a] f32
    p5h = np.ascontiguousarray(
        np.asarray(inputs["p5_w"], dtype=np.float32).reshape(2, 128).T
    ).astype(bf16)
    xs = x.reshape(NCORES, NPER, C, HW).astype(bf16)
    return [
        {
            "x": np.ascontiguousarray(xs[i]),
            "p1": p1h, "wt": wth, "p4": p4h, "p5": p5h,
        }
        for i in range(NCORES)
    ]


def _get_exec():
    """Compile the 8-core PJRT executable once; reuse across kernel() calls."""
    if "exec" in _CACHE:
        return _CACHE["exec"]
    import jax
    from jax.sharding import Mesh, NamedSharding, PartitionSpec
    from jax.experimental.shard_map import shard_map
    import concourse.mybir as mybir
    from concourse.bass2jax import (
        _bass_exec_p,
        install_neuronx_cc_hook,
        partition_id_tensor,
    )

    nc = _get_nc()
    install_neuronx_cc_hook()

    partition_name = (
        nc.partition_id_tensor.name if nc.partition_id_tensor else None
    )
    in_names, out_names, out_avals, zero_shapes = [], [], [], []
    for alloc in nc.m.functions[0].allocations:
        if not isinstance(alloc, mybir.MemoryLocationSet):
            continue
        name = alloc.memorylocations[0].name
        if alloc.kind == "ExternalInput":
            if name != partition_name:
                in_names.append(name)
        elif alloc.kind == "ExternalOutput":
            shape = tuple(alloc.tensor_shape)
            dtype = mybir.dt.np(alloc.dtype)
            out_avals.append(jax.core.ShapedArray(shape, dtype))
            out_names.append(name)
            zero_shapes.append((shape, dtype))
    n_params = len(in_names)
    all_in_names = list(in_names) + list(out_names)
    if partition_name is not None:
        all_in_names.append(partition_name)

    def _body(*args):
        operands = list(args)
        if partition_name is not None:
            operands.append(partition_id_tensor())
        outs = _bass_exec_p.bind(
            *operands,
            out_avals=tuple(out_avals),
            in_names=tuple(all_in_names),
            out_names=tuple(out_names),
            lowering_input_output_aliases=(),
            sim_require_finite=True,
            sim_require_nnan=True,
            nc=nc,
        )
        return tuple(outs)

    devices = jax.devices()[:NCORES]
    mesh = Mesh(np.asarray(devices), ("core",))
    nspecs = n_params + len(out_names)
    fn = jax.jit(
        shard_map(
            _body,
            mesh=mesh,
            in_specs=(PartitionSpec("core"),) * nspecs,
            out_specs=(PartitionSpec("core"),) * len(out_names),
            check_rep=False,
        ),
        keep_unused=True,
    )
    sharding = NamedSharding(mesh, PartitionSpec("core"))
    _CACHE["exec"] = (fn, in_names, out_names, out_avals, zero_shapes, sharding)
    return _CACHE["exec"]


def _run_fast(in_maps):
    """Cached sharded-PJRT executable: no retrace/recompile on repeat calls."""
    import jax

    fn, in_names, out_names, out_avals, zero_shapes, sharding = _get_exec()
    concat_in = [
        np.concatenate([m[nm] for m in in_maps], axis=0) for nm in in_names
    ]
    concat_zeros = [
        np.zeros((NCORES * s[0], *s[1:]), d) for (s, d) in zero_shapes
    ]
    dargs = [jax.device_put(a, sharding) for a in concat_in + concat_zeros]
    out_arrs = fn(*dargs)
    return np.asarray(out_arrs[0], dtype=np.float32)  # (N, 2, 128, HW)


def _run_spmd(in_maps):
    """Portable path via bass_utils (works on native-NRT machines too)."""
    from concourse.bass_utils import run_bass_kernel_spmd

    res = run_bass_kernel_spmd(
        _get_nc(), in_maps, core_ids=list(range(NCORES))
    )
    return np.concatenate(
        [np.asarray(res.results[i]["out"], dtype=np.float32)
         for i in range(NCORES)],
        axis=0,
    )


def kernel(**inputs):
    in_maps = host_prep(inputs)
    if _CACHE.get("fast_ok", True):
        try:
            out = _run_fast(in_maps)
            return out.reshape(N, C, H, W)
        except Exception:
            _CACHE["fast_ok"] = False
    out = _run_spmd(in_maps)
    return out.reshape(N, C, H, W)


# revision 12
# speedup vs baseline: 1.0250x; 1.0250x over previous
"""Trainium2 Bass kernel for the fused einsum/groupconv/bmm module (v2).

Math (per image n, C=256, H=W=56, HW=3136):
  t1[c,e] = sum_s X[c,s] P[s,e]
  t3      = groupconv3x3(x[n], conv_w, groups=2)
  t4      = p4 * t1;  t5[a] = sum_b t4[a,b] p5[b]
  t6      = (t4 @ t3) / 16;  t7[s] = (sum_c t5[c] X[c,s]) / 16
  out     = t6 + t7[broadcast over c]

v2 design (8 cores, 4 images each, bf16 datapath, fp32 PSUM accumulate):
  - All HBM I/O in bf16 with contiguous per-partition layouts (host does
    every relayout): big DMA descriptors only, ~2x fewer wire bytes.
  - X^T built by XBAR dma_start_transpose (2 per image) instead of PE
    transposes + PSUM copies; contiguous dests only (strided XBAR dests
    are wrong on HW).
  - Padded conv layout built on-chip: pad cells memset once per physical
    buffer, interior refreshed per image by one strided copy per c-block.
  - t7 computed once per chunk as an M=1 matmul row, replicated across
    partitions by a gpsimd broadcast DMA (DRAM round-trip; stride-0
    partition AP), and added during the PSUM extraction on DVE — saving
    two rank-128 PE matmuls per (chunk, half); 1/sqrt(C) folded into p4
    host-side.
  - Outputs accumulated into a whole-image SBUF tile, one DMA per
    (image, channel-half); the last image's store is split so the kernel
    tail doesn't serialize behind the full image.
"""

import sys

sys.path.insert(0, "/opt/trn_rl_repo")

import numpy as np

N, C, H, W = 32, 256, 56, 56
HW = H * W            # 3136
PH = H + 2            # 58
PHW = PH * PH         # 3364
XLEN = PHW + 2        # +2 tail guard for the last chunk's corner taps
NCORES = 8
NPER = N // NCORES    # 4 images per core
CHP = 8 * PH          # conv chunk: 8 padded rows = 464 (fits a PSUM bank)
CHI = 8 * W           # interior chunk = 448
NCHUNK = 7
SPAD = 3200           # s padded to 25*128 for XBAR/t1 chunking
KT = SPAD // 128      # 25
INV = 1.0 / 16.0      # 1/sqrt(C)


def build_body(tc, outs, ins):
    import concourse.bass as bass
    import concourse.mybir as mybir

    nc = tc.nc
    f32 = mybir.dt.float32
    bf16 = mybir.dt.bfloat16

    # DRAM scratch ring for the t7 partition-broadcast round-trip
    t7sc_d = nc.dram_tensor("t7sc", (8, CHI), bf16, kind="Internal").ap()

    x_d = ins["x"]          # (NPER, C, HW)        bf16
    p1_d = ins["p1"]        # (128, KT, C)         bf16  [p,k,e] = p1[k*128+p, e]
    wt_d = ins["wt"]        # (128, 2, 9, 128)     bf16  [i,g,t,o]
    p4_d = ins["p4"]        # (128, 2, C)          f32   [a_lo,ab,b] = p4[ab*128+a_lo, b]/16
    p5_d = ins["p5"]        # (128, 2)             bf16  [b,bb] = p5[bb*128+b]
    out_d = outs["out"]     # (NPER, 2, 128, HW)   bf16

    with (
        tc.tile_pool(name="const", bufs=1) as constp,
        tc.tile_pool(name="xtp", bufs=2) as xtp,
        tc.tile_pool(name="svp", bufs=2) as svp,
        tc.tile_pool(name="t3p", bufs=16) as t3p,
        tc.tile_pool(name="outp", bufs=2) as outp,
        tc.tile_pool(name="ps_acc", bufs=2, space="PSUM") as ps_acc,
        tc.tile_pool(name="ps_cv", bufs=2, space="PSUM") as ps_cv,
        tc.tile_pool(name="ps_t6", bufs=2, space="PSUM") as ps_t6,
    ):
        # ---- constants (loaded on the Activation DMA queue so image 0's
        # x loads own the SP queue from t=0; wt first since conv needs it) ----
        p1_sb = constp.tile([128, KT * C], bf16, name="p1_sb")
        wt_sb = constp.tile([128, 2 * 9 * 128], bf16, name="wt_sb")
        p4_sb = constp.tile([128, 2 * C], f32, name="p4_sb")
        p5_sb = constp.tile([128, 2], bf16, name="p5_sb")
        wt_flat = wt_d.rearrange("i g t o -> i (g t o)")
        nc.scalar.dma_start(out=wt_sb[:, 0:1152], in_=wt_flat[:, 0:1152])
        nc.scalar.dma_start(out=wt_sb[:, 1152:2304], in_=wt_flat[:, 1152:2304])
        nc.scalar.dma_start(
            out=p4_sb[:, :], in_=p4_d.rearrange("b bb a -> b (bb a)")
        )
        nc.scalar.dma_start(out=p5_sb[:, :], in_=p5_d)

        # ---- persistent x buffers (explicit A/B double buffering) ----
        # xunp: contiguous x + 64 zero tail cols (zeroed once per buffer).
        # xpad: 58x58 zero-padded layout (+2 guard); pads memset once per
        # buffer, interior rewritten per image.
        xunp = [[None, None], [None, None]]
        xpad = [[None, None], [None, None]]
        for buf in range(2):
            for cb in range(2):
                xu = constp.tile([128, SPAD], bf16, name=f"xu{buf}{cb}")
                nc.vector.memset(xu[:, HW:SPAD], 0.0)
                xunp[buf][cb] = xu
                xp = constp.tile([128, XLEN], bf16, name=f"xq{buf}{cb}")
                # zero only the pad cells (head guard + row 0, the 56 row
                # seams, row 57 + tail guard); interior is rewritten per image
                nc.gpsimd.memset(xp[:, 0:59], 0.0)
                nc.gpsimd.memset(
                    xp[:, 58 : 58 + 57 * PH]
                    .rearrange("p (r w) -> p r w", w=PH)[:, :, 0:2],
                    0.0,
                )
                nc.gpsimd.memset(xp[:, 1 + 57 * PH : XLEN], 0.0)
                xpad[buf][cb] = xp

        def pad_copy(eng, buf, cb, r0, r1):
            """Copy x rows [r0,r1) into the padded interior (offset-1 flat
            layout: head guard cell keeps conv tap offsets >= 0)."""
            copy_fn = eng.copy if eng is nc.scalar else eng.tensor_copy
            copy_fn(
                xpad[buf][cb][:, 1 : 1 + PHW]
                .rearrange("p (r w) -> p r w", w=PH)[:, 1 + r0 : 1 + r1, 1:57],
                xunp[buf][cb][:, r0 * W : r1 * W]
                .rearrange("p (r w) -> p r w", w=W),
            )

        def load_image(n):
            """DMA x[n] in, build X^T chunks and padded layout."""
            buf = n % 2
            if n == 0:
                # cold start: land the first 10 rows of each c-block early so
                # conv chunk 0 can start ~1.3us in instead of ~5us
                for cb in range(2):
                    nc.sync.dma_start(
                        out=xunp[buf][cb][:, 0 : 10 * W],
                        in_=x_d[n, cb * 128 : (cb + 1) * 128, 0 : 10 * W],
                    )
                pad_copy(nc.vector, buf, 0, 0, 10)
                pad_copy(nc.scalar, buf, 1, 0, 10)
                for cb in range(2):
                    nc.sync.dma_start(
                        out=xunp[buf][cb][:, 10 * W : HW],
                        in_=x_d[n, cb * 128 : (cb + 1) * 128, 10 * W : HW],
                    )
                pad_copy(nc.vector, buf, 0, 10, 56)
                pad_copy(nc.scalar, buf, 1, 10, 56)
            else:
                for cb in range(2):
                    nc.sync.dma_start(
                        out=xunp[buf][cb][:, 0:HW],
                        in_=x_d[n, cb * 128 : (cb + 1) * 128, :],
                    )
                pad_copy(nc.vector, buf, 0, 0, 56)
                pad_copy(nc.gpsimd, buf, 1, 0, 56)
            xts = []
            for cb in range(2):
                xt = xtp.tile([128, KT * 128], bf16, tag=f"xt{cb}",
                              name=f"xt{cb}_{n}")
                nc.sync.dma_start_transpose(
                    xt.rearrange("p (k e) -> p k e", e=128),
                    xunp[buf][cb][:, :],
                )
                xts.append(xt)
            return xts

        xts_cur = load_image(0)
        # p1 rides the SP queue behind image 0's loads/XBARs; t1 needs it
        # only after image 0's conv block (~29us in)
        nc.sync.dma_start(
            out=p1_sb[:, :], in_=p1_d.rearrange("p k e -> p (k e)")
        )

        def t14(n, xts):
            """t1 (untransposed) -> t4 -> XBAR-transposed t4T blocks.

            t1'[c,e] = sum_s x[c,s] p1[s,e] via lhsT = X^T chunks; then
            t4' = p4/16 * t1' elementwise, and t4T[ab][b_lo, bb, a_lo] =
            t4'[ab][a_lo, bb*128+b_lo] via one XBAR transpose per a-block.
            """
            t4Ts = []
            t4ps = []
            for cb in range(2):
                t1ps = ps_acc.tile([128, C], f32, tag="t1", name=f"t1ps{cb}")
                for k in range(KT):
                    nc.tensor.matmul(
                        t1ps[:, :],
                        xts[cb][:, k * 128 : (k + 1) * 128],
                        p1_sb[:, k * C : (k + 1) * C],
                        start=(k == 0),
                        stop=(k == KT - 1),
                    )
                t4p = svp.tile([128, C], bf16, tag="t4p", name=f"t4p{cb}_{n}")
                nc.vector.tensor_mul(
                    t4p[:, :], t1ps[:, :], p4_sb[:, cb * C : (cb + 1) * C]
                )
                t4T = svp.tile([128, C], bf16, tag="t4T", bufs=4,
                               name=f"t4T{cb}_{n}")
                nc.sync.dma_start_transpose(
                    t4T.rearrange("p (kb a) -> p kb a", a=128), t4p[:, :]
                )
                t4Ts.append(t4T)
                t4ps.append(t4p)
            return t4Ts

        def t5part(n, t4Ts):
            """t5 column and broadcast-t5 lhsT (PE matmuls; the DVE
            tensor_tensor_reduce variant wedges real hardware)."""
            t5ps = ps_acc.tile([128, 2], f32, tag="t7p", bufs=2,
                               name=f"t5ps_{n}")
            for ab in range(2):
                for bb in range(2):
                    nc.tensor.matmul(
                        t5ps[:, ab : ab + 1],
                        t4Ts[ab][:, bb * 128 : (bb + 1) * 128],
                        p5_sb[:, bb : bb + 1],
                        start=(bb == 0),
                        stop=(bb == 1),
                    )
            t5col = svp.tile([128, 2], bf16, tag="t5c", name=f"t5col_{n}")
            nc.scalar.copy(t5col[:, :], t5ps[:, :])
            return t5col

        def t7make(n, c, t5col, buf):
            """t7 row for chunk c, replicated across partitions off the PE.

            Two M=1 matmuls build t7row [1,448] in PSUM; it round-trips
            through a DRAM scratch slot so a gpsimd (software-DGE) DMA with
            a stride-0 partition AP can replicate it to all 128 partitions —
            the add into t6 then rides the existing extraction copy on DVE
            instead of two rank-128 PE matmuls per (chunk, ab).
            """
            t7ps = ps_acc.tile([1, CHI], f32, tag="t7p", bufs=2,
                               name=f"t7ps_{n}_{c}")
            for cb in range(2):
                nc.tensor.matmul(
                    t7ps[:, :],
                    t5col[:, cb : cb + 1],
                    xunp[buf][cb][:, c * CHI : (c + 1) * CHI],
                    start=(cb == 0),
                    stop=(cb == 1),
                )
            t7row = svp.tile([1, CHI], bf16, tag="t7r", bufs=2,
                             name=f"t7row_{n}_{c}")
            nc.scalar.copy(t7row[:, :], t7ps[:, :])
            slot = (n * NCHUNK + c) % 8
            scr = t7sc_d[slot : slot + 1, :]
            nc.sync.dma_start(out=scr, in_=t7row[:, :])
            t7rep = svp.tile([128, CHI], bf16, tag="t7rep", bufs=6,
                             name=f"t7rep_{n}_{c}")
            nc.gpsimd.dma_start(
                out=t7rep[:, :],
                in_=bass.AP(
                    tensor=scr.tensor,
                    offset=scr.offset,
                    ap=[[0, 128]] + list(scr.ap[1:]),
                ),
            )
            return t7rep

        for n in range(NPER):
            buf = n % 2

            if n > 0:
                t4Ts = t14(n, xts_cur)

            # prefetch next image while PE runs this image's chunk loop
            xts_next = load_image(n + 1) if n + 1 < NPER else None

            # ---- chunk loop: conv(c) then t6(c-4), software-pipelined ----
            obig = outp.tile([128, 2 * HW], bf16, tag="ob", name=f"ob_{n}")
            t3cs = {}
            t7reps = {}

            def conv(c):
                r0 = 1 + 8 * c
                pair = []
                for g in range(2):
                    cv = ps_cv.tile([128, CHP], f32, tag="cv",
                                    name=f"cv{g}_{n}_{c}")
                    for tap in range(9):
                        kh, kw = tap // 3, tap % 3
                        foff = (r0 + kh - 1) * PH + kw
                        nc.tensor.matmul(
                            cv[:, :],
                            wt_sb[:, (g * 9 + tap) * 128 : (g * 9 + tap) * 128 + 128],
                            xpad[buf][g][:, foff : foff + CHP],
                            start=(tap == 0),
                            stop=(tap == 8),
                        )
                    t3g = t3p.tile([128, CHI], bf16, tag="t3",
                                   name=f"t3g{g}_{n}_{c}")
                    eng = nc.vector if g == 0 else nc.scalar
                    eng_copy = (eng.tensor_copy if g == 0 else eng.copy)
                    eng_copy(
                        t3g.rearrange("p (r w) -> p r w", w=W),
                        cv.rearrange("p (r w) -> p r w", w=PH)[:, :, 1:57],
                    )
                    pair.append(t3g)
                t3cs[c] = pair

            def t6(c):
                pair = t3cs.pop(c)
                t7rep = t7reps.pop(c)
                for ab in range(2):
                    t6ps = ps_t6.tile([128, CHI], f32, tag="t6",
                                      name=f"t6ps{ab}_{n}_{c}")
                    nc.tensor.matmul(
                        t6ps[:, :],
                        t4Ts[ab][:, 0:128],
                        pair[0][:, :],
                        start=True, stop=False,
                    )
                    nc.tensor.matmul(
                        t6ps[:, :],
                        t4Ts[ab][:, 128:256],
                        pair[1][:, :],
                        start=False, stop=True,
                    )
                    # t7 broadcast-add fused into the extraction (DVE only:
                    # Pool can't read PSUM, Act can't add two tensors)
                    nc.vector.tensor_add(
                        obig[:, ab * HW + c * CHI : ab * HW + (c + 1) * CHI],
                        t6ps[:, :],
                        t7rep[:, :],
                    )

            def flush(lo, hi):
                # flush finished output chunks early so the final store
                # doesn't serialize behind the whole image
                for ab in range(2):
                    nc.sync.dma_start(
                        out=out_d[n, ab][:, lo * CHI : hi * CHI],
                        in_=obig[:, ab * HW + lo * CHI : ab * HW + hi * CHI],
                    )

            # split the store only on the last image, where it shortens the
            # kernel tail; elsewhere one whole-buffer DMA per ab is cheaper
            last = n == NPER - 1

            if n == 0:
                # cold start: conv only needs the padded x (ready ~5us in),
                # while t1 waits on the XBAR transposes — run conv first
                for c in range(NCHUNK):
                    conv(c)
                t4Ts = t14(n, xts_cur)
                t5col = t5part(n, t4Ts)
                for c in range(3):
                    t7reps[c] = t7make(n, c, t5col, buf)
                for c in range(NCHUNK):
                    if c + 3 < NCHUNK:
                        t7reps[c + 3] = t7make(n, c + 3, t5col, buf)
                    t6(c)
                    if last and c == 4:
                        flush(0, 4)
                    elif last and c == 5:
                        flush(4, 6)
            else:
                next_t7 = 0
                for c in range(NCHUNK):
                    conv(c)
                    if c == 2:
                        # t5/t6 consume the XBAR'd t4T; two conv chunks
                        # (~7us) of slack in case real XBAR latency exceeds
                        # the cost model's 14ns/tile
                        t5col = t5part(n, t4Ts)
                    elif c >= 3:
                        for _ in range(2):
                            if next_t7 < NCHUNK:
                                t7reps[next_t7] = t7make(
                                    n, next_t7, t5col, buf
                                )
                                next_t7 += 1
                        if c >= 4:
                            t6(c - 4)
                if last:
                    flush(0, 3)
                for c in range(3, NCHUNK):
                    t6(c)
                    if last and c == 5:
                        flush(3, 6)

            if last:
                flush(6, NCHUNK)
            else:
                flush(0, NCHUNK)
            xts_cur = xts_next


_CACHE = {}


def _get_nc():
    if "nc" in _CACHE:
        return _CACHE["nc"]
    import concourse.bacc as bacc
    import concourse.mybir as mybir
    import concourse.tile as tile

    f32 = mybir.dt.float32
    bf16 = mybir.dt.bfloat16
    nc = bacc.Bacc(
        "TRN2", target_bir_lowering=False, debug=False, num_devices=NCORES
    )
    ins = {
        "x": nc.dram_tensor("x", (NPER, C, HW), bf16, kind="ExternalInput").ap(),
        "p1": nc.dram_tensor("p1", (128, KT, C), bf16, kind="ExternalInput").ap(),
        "wt": nc.dram_tensor("wt", (128, 2, 9, 128), bf16, kind="ExternalInput").ap(),
        "p4": nc.dram_tensor("p4", (128, 2, C), f32, kind="ExternalInput").ap(),
        "p5": nc.dram_tensor("p5", (128, 2), bf16, kind="ExternalInput").ap(),
    }
    outs = {
        "out": nc.dram_tensor(
            "out", (NPER, 2, 128, HW), bf16, kind="ExternalOutput"
        ).ap(),
    }
    with tile.TileContext(nc) as tc:
        build_body(tc, outs, ins)
    nc.compile()
    _CACHE["nc"] = nc
    return nc


def host_prep(inputs):
    """Split full inputs into per-core in_maps (host-side relayout + bf16)."""
    import ml_dtypes

    bf16 = ml_dtypes.bfloat16
    x = np.asarray(inputs["x"], dtype=np.float32).reshape(N, C, HW)
    p1 = np.asarray(inputs["p1_w"], dtype=np.float32)[..., 0].reshape(HW, C)
    p1p = np.zeros((SPAD, C), dtype=np.float32)
    p1p[0:HW] = p1
    p1h = np.ascontiguousarray(
        p1p.reshape(KT, 128, C).transpose(1, 0, 2)
    ).astype(bf16)
    wt = np.asarray(inputs["conv_w"], dtype=np.float32)  # (256, 128, 3, 3)
    wth = np.ascontiguousarray(
        wt.reshape(2, 128, 128, 9).transpose(2, 0, 3, 1)
    ).astype(bf16)  # [i, g, t, o]
    p4 = np.asarray(inputs["p4_w"], dtype=np.float32)[0]  # (a, b)
    p4h = np.ascontiguousarray(
        (p4 * INV).reshape(2, 128, C).transpose(1, 0, 2)
    )  # [a_lo, ab, b]  # [b, bb, a] f32
    p5h = np.ascontiguousarray(
        np.asarray(inputs["p5_w"], dtype=np.float32).reshape(2, 128).T
    ).astype(bf16)
    xs = x.reshape(NCORES, NPER, C, HW).astype(bf16)
    return [
        {
            "x": np.ascontiguousarray(xs[i]),
            "p1": p1h, "wt": wth, "p4": p4h, "p5": p5h,
        }
        for i in range(NCORES)
    ]


def _get_exec():
    """Compile the 8-core PJRT executable once; reuse across kernel() calls."""
    if "exec" in _CACHE:
        return _CACHE["exec"]
    import jax
    from jax.sharding import Mesh, NamedSharding, PartitionSpec
    from jax.experimental.shard_map import shard_map
    import concourse.mybir as mybir
    from concourse.bass2jax import (
        _bass_exec_p,
        install_neuronx_cc_hook,
        partition_id_tensor,
    )

    nc = _get_nc()
    install_neuronx_cc_hook()

    partition_name = (
        nc.partition_id_tensor.name if nc.partition_id_tensor else None
    )
    in_names, out_names, out_avals, zero_shapes = [], [], [], []
    for alloc in nc.m.functions[0].allocations:
        if not isinstance(alloc, mybir.MemoryLocationSet):
            continue
        name = alloc.memorylocations[0].name
        if alloc.kind == "ExternalInput":
            if name != partition_name:
                in_names.append(name)
        elif alloc.kind == "ExternalOutput":
            shape = tuple(alloc.tensor_shape)
            dtype = mybir.dt.np(alloc.dtype)
            out_avals.append(jax.core.ShapedArray(shape, dtype))
            out_names.append(name)
            zero_shapes.append((shape, dtype))
    n_params = len(in_names)
    all_in_names = list(in_names) + list(out_names)
    if partition_name is not None:
        all_in_names.append(partition_name)

    def _body(*args):
        operands = list(args)
        if partition_name is not None:
            operands.append(partition_id_tensor())
        outs = _bass_exec_p.bind(
            *operands,
            out_avals=tuple(out_avals),
            in_names=tuple(all_in_names),
            out_names=tuple(out_names),
            lowering_input_output_aliases=(),
            sim_require_finite=True,
            sim_require_nnan=True,
            nc=nc,
        )
        return tuple(outs)

    devices = jax.devices()[:NCORES]
    mesh = Mesh(np.asarray(devices), ("core",))
    nspecs = n_params + len(out_names)
    fn = jax.jit(
        shard_map(
            _body,
            mesh=mesh,
            in_specs=(PartitionSpec("core"),) * nspecs,
            out_specs=(PartitionSpec("core"),) * len(out_names),
            check_rep=False,
        ),
        keep_unused=True,
    )
    sharding = NamedSharding(mesh, PartitionSpec("core"))
    _CACHE["exec"] = (fn, in_names, out_names, out_avals, zero_shapes, sharding)
    return _CACHE["exec"]


def _run_fast(in_maps):
    """Cached sharded-PJRT executable: no retrace/recompile on repeat calls."""
    import jax

    fn, in_names, out_names, out_avals, zero_shapes, sharding = _get_exec()
    concat_in = [
        np.concatenate([m[nm] for m in in_maps], axis=0) for nm in in_names
    ]
    concat_zeros = [
        np.zeros((NCORES * s[0], *s[1:]), d) for (s, d) in zero_shapes
    ]
    dargs = [jax.device_put(a, sharding) for a in concat_in + concat_zeros]
    out_arrs = fn(*dargs)
    return np.asarray(out_arrs[0], dtype=np.float32)  # (N, 2, 128, HW)


def _run_spmd(in_maps):
    """Portable path via bass_utils (works on native-NRT machines too)."""
    from concourse.bass_utils import run_bass_kernel_spmd

    res = run_bass_kernel_spmd(
        _get_nc(), in_maps, core_ids=list(range(NCORES))
    )
    return np.concatenate(
        [np.asarray(res.results[i]["out"], dtype=np.float32)
         for i in range(NCORES)],
        axis=0,
    )


def kernel(**inputs):
    in_maps = host_prep(inputs)
    if _CACHE.get("fast_ok", True):
        try:
            out = _run_fast(in_maps)
            return out.reshape(N, C, H, W)
        except Exception:
            _CACHE["fast_ok"] = False
    out = _run_spmd(in_maps)
    return out.reshape(N, C, H, W)


# revision 13
# speedup vs baseline: 1.1820x; 1.1531x over previous
"""Trainium2 Bass kernel for the fused einsum/groupconv/bmm module (v2).

Math (per image n, C=256, H=W=56, HW=3136):
  t1[c,e] = sum_s X[c,s] P[s,e]
  t3      = groupconv3x3(x[n], conv_w, groups=2)
  t4      = p4 * t1;  t5[a] = sum_b t4[a,b] p5[b]
  t6      = (t4 @ t3) / 16;  t7[s] = (sum_c t5[c] X[c,s]) / 16
  out     = t6 + t7[broadcast over c]

v2 design (8 cores, 4 images each, bf16 datapath, fp32 PSUM accumulate):
  - All HBM I/O in bf16 with contiguous per-partition layouts (host does
    every relayout): big DMA descriptors only, ~2x fewer wire bytes.
  - X^T built by XBAR dma_start_transpose (2 per image) instead of PE
    transposes + PSUM copies; contiguous dests only (strided XBAR dests
    are wrong on HW).
  - Padded conv layout built on-chip: pad cells memset once per physical
    buffer, interior refreshed per image by one strided copy per c-block.
  - t7 computed once per chunk as an M=1 matmul row, replicated across
    partitions by a gpsimd broadcast DMA (DRAM round-trip; stride-0
    partition AP), and added during the PSUM extraction on DVE — saving
    two rank-128 PE matmuls per (chunk, half); 1/sqrt(C) folded into p4
    host-side.
  - Outputs accumulated into a whole-image SBUF tile, one DMA per
    (image, channel-half); the last image's store is split so the kernel
    tail doesn't serialize behind the full image.
"""

import sys

sys.path.insert(0, "/opt/trn_rl_repo")

import numpy as np

N, C, H, W = 32, 256, 56, 56
HW = H * W            # 3136
PH = H + 2            # 58
PHW = PH * PH         # 3364
XLEN = PHW + 2        # +2 tail guard for the last chunk's corner taps
NCORES = 8
NPER = N // NCORES    # 4 images per core
CHP = 8 * PH          # conv chunk: 8 padded rows = 464 (fits a PSUM bank)
CHI = 8 * W           # interior chunk = 448
NCHUNK = 7
SPAD = 3200           # s padded to 25*128 for XBAR/t1 chunking
KT = SPAD // 128      # 25
INV = 1.0 / 16.0      # 1/sqrt(C)


def build_body(tc, outs, ins):
    import concourse.bass as bass
    import concourse.mybir as mybir

    nc = tc.nc
    f32 = mybir.dt.float32
    bf16 = mybir.dt.bfloat16

    # DRAM scratch for the t5/t7 partition-broadcast round-trips (double
    # buffered across images)
    t5sc_d = nc.dram_tensor("t5sc", (2, 2, 128), bf16, kind="Internal").ap()
    t7fu_d = nc.dram_tensor("t7fu", (2, KT, 128), bf16, kind="Internal").ap()

    x_d = ins["x"]          # (NPER, C, HW)        bf16
    p1_d = ins["p1"]        # (128, KT, C)         bf16  [p,k,e] = p1[k*128+p, e]
    wt_d = ins["wt"]        # (128, 2, 9, 128)     bf16  [i,g,t,o]
    p4_d = ins["p4"]        # (128, 2, C)          f32   [a_lo,ab,b] = p4[ab*128+a_lo, b]/16
    p5_d = ins["p5"]        # (1, C)               bf16  p5 flat row
    out_d = outs["out"]     # (NPER, 2, 128, HW)   bf16

    with (
        tc.tile_pool(name="const", bufs=1) as constp,
        tc.tile_pool(name="xtp", bufs=2) as xtp,
        tc.tile_pool(name="svp", bufs=2) as svp,
        tc.tile_pool(name="t3p", bufs=16) as t3p,
        tc.tile_pool(name="outp", bufs=2) as outp,
        tc.tile_pool(name="ps_acc", bufs=2, space="PSUM") as ps_acc,
        tc.tile_pool(name="ps_cv", bufs=2, space="PSUM") as ps_cv,
        tc.tile_pool(name="ps_t6", bufs=4, space="PSUM") as ps_t6,
    ):
        # ---- constants (loaded on the Activation DMA queue so image 0's
        # x loads own the SP queue from t=0; wt first since conv needs it) ----
        p1_sb = constp.tile([128, KT * C], bf16, name="p1_sb")
        wt_sb = constp.tile([128, 2 * 9 * 128], bf16, name="wt_sb")
        p4_sb = constp.tile([128, 2 * C], f32, name="p4_sb")
        p5r_sb = constp.tile([128, C], bf16, name="p5r_sb")
        wt_flat = wt_d.rearrange("i g t o -> i (g t o)")
        nc.scalar.dma_start(out=wt_sb[:, 0:1152], in_=wt_flat[:, 0:1152])
        nc.scalar.dma_start(out=wt_sb[:, 1152:2304], in_=wt_flat[:, 1152:2304])
        nc.scalar.dma_start(
            out=p4_sb[:, :], in_=p4_d.rearrange("b bb a -> b (bb a)")
        )
        nc.gpsimd.dma_start(
            out=p5r_sb[:, :],
            in_=bass.AP(tensor=p5_d.tensor, offset=p5_d.offset,
                        ap=[[0, 128]] + list(p5_d.ap[1:])),
        )

        # ---- persistent x buffers (explicit A/B double buffering) ----
        # xunp: contiguous x + 64 zero tail cols (zeroed once per buffer).
        # xpad: 58x58 zero-padded layout (+2 guard); pads memset once per
        # buffer, interior rewritten per image.
        xunp = [[None, None], [None, None]]
        xpad = [[None, None], [None, None]]
        for buf in range(2):
            for cb in range(2):
                xu = constp.tile([128, SPAD], bf16, name=f"xu{buf}{cb}")
                nc.vector.memset(xu[:, HW:SPAD], 0.0)
                xunp[buf][cb] = xu
                xp = constp.tile([128, XLEN], bf16, name=f"xq{buf}{cb}")
                # zero only the pad cells (head guard + row 0, the 56 row
                # seams, row 57 + tail guard); interior is rewritten per image
                nc.gpsimd.memset(xp[:, 0:59], 0.0)
                nc.gpsimd.memset(
                    xp[:, 58 : 58 + 57 * PH]
                    .rearrange("p (r w) -> p r w", w=PH)[:, :, 0:2],
                    0.0,
                )
                nc.gpsimd.memset(xp[:, 1 + 57 * PH : XLEN], 0.0)
                xpad[buf][cb] = xp

        def pad_copy(eng, buf, cb, r0, r1):
            """Copy x rows [r0,r1) into the padded interior (offset-1 flat
            layout: head guard cell keeps conv tap offsets >= 0)."""
            copy_fn = eng.copy if eng is nc.scalar else eng.tensor_copy
            copy_fn(
                xpad[buf][cb][:, 1 : 1 + PHW]
                .rearrange("p (r w) -> p r w", w=PH)[:, 1 + r0 : 1 + r1, 1:57],
                xunp[buf][cb][:, r0 * W : r1 * W]
                .rearrange("p (r w) -> p r w", w=W),
            )

        def load_image(n):
            """DMA x[n] in, build X^T chunks and padded layout."""
            buf = n % 2
            if n == 0:
                # cold start: land the first 10 rows of each c-block early so
                # conv chunk 0 can start ~1.3us in instead of ~5us
                for cb in range(2):
                    nc.sync.dma_start(
                        out=xunp[buf][cb][:, 0 : 10 * W],
                        in_=x_d[n, cb * 128 : (cb + 1) * 128, 0 : 10 * W],
                    )
                pad_copy(nc.vector, buf, 0, 0, 10)
                pad_copy(nc.scalar, buf, 1, 0, 10)
                for cb in range(2):
                    nc.sync.dma_start(
                        out=xunp[buf][cb][:, 10 * W : HW],
                        in_=x_d[n, cb * 128 : (cb + 1) * 128, 10 * W : HW],
                    )
                pad_copy(nc.vector, buf, 0, 10, 56)
                pad_copy(nc.scalar, buf, 1, 10, 56)
            else:
                for cb in range(2):
                    nc.sync.dma_start(
                        out=xunp[buf][cb][:, 0:HW],
                        in_=x_d[n, cb * 128 : (cb + 1) * 128, :],
                    )
                pad_copy(nc.vector, buf, 0, 0, 56)
                pad_copy(nc.scalar, buf, 1, 0, 56)
            xts = []
            for cb in range(2):
                xt = xtp.tile([128, KT * 128], bf16, tag=f"xt{cb}",
                              name=f"xt{cb}_{n}")
                nc.sync.dma_start_transpose(
                    xt.rearrange("p (k e) -> p k e", e=128),
                    xunp[buf][cb][:, :],
                )
                xts.append(xt)
            return xts

        xts_cur = load_image(0)
        # p1 rides the SP queue behind image 0's loads/XBARs; t1 needs it
        # only after image 0's conv block (~29us in)
        nc.sync.dma_start(
            out=p1_sb[:, :], in_=p1_d.rearrange("p k e -> p (k e)")
        )

        def t14(n, xts):
            """t1 (untransposed) -> t4 -> XBAR-transposed t4T blocks.

            t1'[c,e] = sum_s x[c,s] p1[s,e] via lhsT = X^T chunks; then
            t4' = p4/16 * t1' elementwise, and t4T[ab][b_lo, bb, a_lo] =
            t4'[ab][a_lo, bb*128+b_lo] via one XBAR transpose per a-block.
            """
            t4Ts = []
            t4ps = []
            for cb in range(2):
                t1ps = ps_acc.tile([128, C], f32, tag="t1", name=f"t1ps{cb}")
                for k in range(KT):
                    nc.tensor.matmul(
                        t1ps[:, :],
                        xts[cb][:, k * 128 : (k + 1) * 128],
                        p1_sb[:, k * C : (k + 1) * C],
                        start=(k == 0),
                        stop=(k == KT - 1),
                    )
                t4p = svp.tile([128, C], bf16, tag="t4p", name=f"t4p{cb}_{n}")
                nc.vector.tensor_mul(
                    t4p[:, :], t1ps[:, :], p4_sb[:, cb * C : (cb + 1) * C]
                )
                t4T = svp.tile([128, C], bf16, tag="t4T", bufs=4,
                               name=f"t4T{cb}_{n}")
                nc.sync.dma_start_transpose(
                    t4T.rearrange("p (kb a) -> p kb a", a=128), t4p[:, :]
                )
                t4Ts.append(t4T)
                t4ps.append(t4p)
            return t4Ts, t4ps

        def t5part(n, t4ps):
            """t5 column via DVE mult+reduce over t4p rows (no PE work)."""
            t5col = svp.tile([128, 2], f32, tag="t5c", name=f"t5col_{n}")
            for cb in range(2):
                scr = svp.tile([128, C], bf16, tag="t5scr",
                               name=f"t5scr{cb}_{n}")
                nc.vector.tensor_mul(scr[:, :], t4ps[cb][:, :], p5r_sb[:, :])
                nc.vector.reduce_sum(
                    t5col[:, cb : cb + 1], scr[:, :],
                    axis=mybir.AxisListType.X,
                )
            t5colb = svp.tile([128, 2], bf16, tag="t5cb", name=f"t5colb_{n}")
            nc.scalar.copy(t5colb[:, :], t5col[:, :])
            slot = n % 2
            for cb in range(2):
                nc.sync.dma_start(
                    out=t5sc_d[slot, cb], in_=t5colb[:, cb : cb + 1]
                )
            return slot

        def t7chain(n, slot, xts):
            """All of t7 off the PE: DVE broadcast-mult + segmented reduce
            over X^T, XBAR flip to row-major, DRAM flatten, then 7
            partition-broadcast chunk tiles."""
            t7T = svp.tile([128, 2 * KT], f32, tag="t7T", name=f"t7T_{n}")
            for cb in range(2):
                t5r = svp.tile([128, 128], bf16, tag="t5r",
                               name=f"t5r{cb}_{n}")
                src = t5sc_d[slot, cb : cb + 1, :]
                nc.gpsimd.dma_start(
                    out=t5r[:, :],
                    in_=bass.AP(tensor=src.tensor, offset=src.offset,
                                ap=[[0, 128], [1, 128]]),
                )
                tmp = svp.tile([128, KT * 128], bf16, tag="t7tmp",
                               name=f"t7tmp{cb}_{n}")
                nc.gpsimd.tensor_mul(
                    tmp.rearrange("p (k c) -> p k c", c=128),
                    xts[cb].rearrange("p (k c) -> p k c", c=128),
                    t5r[:, None, :].broadcast_to([128, KT, 128]),
                )
                nc.vector.reduce_sum(
                    t7T[:, cb * KT : (cb + 1) * KT, None],
                    tmp.rearrange("p (k c) -> p k c", c=128),
                    axis=mybir.AxisListType.X,
                )
            t7Tb = svp.tile([128, 128], bf16, tag="t7Tb", name=f"t7Tb_{n}")
            nc.vector.tensor_add(
                t7Tb[:, 0:KT], t7T[:, 0:KT], t7T[:, KT : 2 * KT]
            )
            nc.vector.memset(t7Tb[:, KT:128], 0.0)
            t7flip = svp.tile([128, 128], bf16, tag="t7fl",
                              name=f"t7flip_{n}")
            nc.sync.dma_start_transpose(t7flip[:, :], t7Tb[:, :])
            nc.sync.dma_start(out=t7fu_d[slot], in_=t7flip[0:KT, :])
            reps = {}
            for c in range(NCHUNK):
                t7rep = svp.tile([128, CHI], bf16, tag="t7rep", bufs=14,
                                 name=f"t7rep_{n}_{c}")
                nc.gpsimd.dma_start(
                    out=t7rep[:, :],
                    in_=bass.AP(
                        tensor=t7fu_d.tensor,
                        offset=t7fu_d.offset + slot * KT * 128 + c * CHI,
                        ap=[[0, 128], [1, CHI]],
                    ),
                )
                reps[c] = t7rep
            return reps

        # ---- flat cross-image chunk pipeline ----
        # conv(n,c) runs at global position 7n+c; t6 retires position-4, so
        # an image's trailing t6s interleave with the NEXT image's conv/t1
        # stream and the DVE drains always have PE work to hide behind.
        ctxs = {}

        def conv_g(n, c):
            ctx = ctxs[n]
            r0 = 1 + 8 * c
            pair = []
            for g in range(2):
                cv = ps_cv.tile([128, CHP], f32, tag="cv",
                                name=f"cv{g}_{n}_{c}")
                for tap in range(9):
                    kh, kw = tap // 3, tap % 3
                    foff = (r0 + kh - 1) * PH + kw
                    nc.tensor.matmul(
                        cv[:, :],
                        wt_sb[:, (g * 9 + tap) * 128 : (g * 9 + tap) * 128 + 128],
                        xpad[ctx["buf"]][g][:, foff : foff + CHP],
                        start=(tap == 0),
                        stop=(tap == 8),
                    )
                t3g = t3p.tile([128, CHI], bf16, tag="t3",
                               name=f"t3g{g}_{n}_{c}")
                eng = nc.vector if g == 0 else nc.scalar
                eng_copy = (eng.tensor_copy if g == 0 else eng.copy)
                eng_copy(
                    t3g.rearrange("p (r w) -> p r w", w=W),
                    cv.rearrange("p (r w) -> p r w", w=PH)[:, :, 1:57],
                )
                pair.append(t3g)
            ctx["t3cs"][c] = pair

        def t6_g(n, c):
            ctx = ctxs[n]
            pair = ctx["t3cs"].pop(c)
            t7rep = ctx["t7reps"].pop(c)
            obig = ctx["obig"]
            for ab in range(2):
                t6ps = ps_t6.tile([128, CHI], f32, tag="t6",
                                  name=f"t6ps{ab}_{n}_{c}")
                nc.tensor.matmul(
                    t6ps[:, :],
                    ctx["t4Ts"][ab][:, 0:128],
                    pair[0][:, :],
                    start=True, stop=False,
                )
                nc.tensor.matmul(
                    t6ps[:, :],
                    ctx["t4Ts"][ab][:, 128:256],
                    pair[1][:, :],
                    start=False, stop=True,
                )
                # t7 broadcast-add fused into the PSUM drain (DVE only)
                nc.vector.tensor_add(
                    obig[:, ab * HW + c * CHI : ab * HW + (c + 1) * CHI],
                    t6ps[:, :],
                    t7rep[:, :],
                )
            if c == NCHUNK - 1 and n < NPER - 1:
                for ab in range(2):
                    nc.sync.dma_start(
                        out=out_d[n, ab],
                        in_=obig[:, ab * HW : (ab + 1) * HW],
                    )

        def flush_last(lo, hi):
            n = NPER - 1
            obig = ctxs[n]["obig"]
            for ab in range(2):
                nc.sync.dma_start(
                    out=out_d[n, ab][:, lo * CHI : hi * CHI],
                    in_=obig[:, ab * HW + lo * CHI : ab * HW + hi * CHI],
                )

        def start_image(n, xts):
            ctxs[n] = {
                "buf": n % 2,
                "obig": outp.tile([128, 2 * HW], bf16, tag="ob",
                                  name=f"ob_{n}"),
                "t3cs": {},
            }
            t4Ts, t4ps = t14(n, xts)
            ctxs[n]["t4Ts"] = t4Ts
            ctxs[n]["t7reps"] = t7chain(n, t5part(n, t4ps), xts)

        xts_next = None
        for n in range(NPER):
            if n == 0:
                # cold start: conv needs only the padded x (~2us in); t1
                # waits on the XBAR, so run three conv chunks first
                ctxs[0] = {"buf": 0, "obig": None, "t3cs": {}}
                for c in range(3):
                    conv_g(0, c)
                saved = ctxs[0]["t3cs"]
                start_image(0, xts_cur)
                ctxs[0]["t3cs"].update(saved)
                xts_next = load_image(1)
                for c in range(3, NCHUNK):
                    conv_g(0, c)
                    if c >= 4:
                        t6_g(0, c - 4)
            else:
                start_image(n, xts_cur)
                if n + 1 < NPER:
                    xts_next = load_image(n + 1)
                for c in range(NCHUNK):
                    conv_g(n, c)
                    p = 7 * n + c - 4
                    t6_g(p // 7, p % 7)
            xts_cur = xts_next

        # drain the last image's trailing chunks
        t6_g(NPER - 1, 3)
        flush_last(0, 4)
        t6_g(NPER - 1, 4)
        t6_g(NPER - 1, 5)
        flush_last(4, 6)
        t6_g(NPER - 1, 6)
        flush_last(6, NCHUNK)


_CACHE = {}


def _get_nc():
    if "nc" in _CACHE:
        return _CACHE["nc"]
    import concourse.bacc as bacc
    import concourse.mybir as mybir
    import concourse.tile as tile

    f32 = mybir.dt.float32
    bf16 = mybir.dt.bfloat16
    nc = bacc.Bacc(
        "TRN2", target_bir_lowering=False, debug=False, num_devices=NCORES
    )
    ins = {
        "x": nc.dram_tensor("x", (NPER, C, HW), bf16, kind="ExternalInput").ap(),
        "p1": nc.dram_tensor("p1", (128, KT, C), bf16, kind="ExternalInput").ap(),
        "wt": nc.dram_tensor("wt", (128, 2, 9, 128), bf16, kind="ExternalInput").ap(),
        "p4": nc.dram_tensor("p4", (128, 2, C), f32, kind="ExternalInput").ap(),
        "p5": nc.dram_tensor("p5", (1, C), bf16, kind="ExternalInput").ap(),
    }
    outs = {
        "out": nc.dram_tensor(
            "out", (NPER, 2, 128, HW), bf16, kind="ExternalOutput"
        ).ap(),
    }
    with tile.TileContext(nc) as tc:
        build_body(tc, outs, ins)
    nc.compile()
    _CACHE["nc"] = nc
    return nc


def host_prep(inputs):
    """Split full inputs into per-core in_maps (host-side relayout + bf16)."""
    import ml_dtypes

    bf16 = ml_dtypes.bfloat16
    x = np.asarray(inputs["x"], dtype=np.float32).reshape(N, C, HW)
    p1 = np.asarray(inputs["p1_w"], dtype=np.float32)[..., 0].reshape(HW, C)
    p1p = np.zeros((SPAD, C), dtype=np.float32)
    p1p[0:HW] = p1
    p1h = np.ascontiguousarray(
        p1p.reshape(KT, 128, C).transpose(1, 0, 2)
    ).astype(bf16)
    wt = np.asarray(inputs["conv_w"], dtype=np.float32)  # (256, 128, 3, 3)
    wth = np.ascontiguousarray(
        wt.reshape(2, 128, 128, 9).transpose(2, 0, 3, 1)
    ).astype(bf16)  # [i, g, t, o]
    p4 = np.asarray(inputs["p4_w"], dtype=np.float32)[0]  # (a, b)
    p4h = np.ascontiguousarray(
        (p4 * INV).reshape(2, 128, C).transpose(1, 0, 2)
    )  # [a_lo, ab, b]  # [b, bb, a] f32
    p5h = np.ascontiguousarray(
        np.asarray(inputs["p5_w"], dtype=np.float32).reshape(1, C)
    ).astype(bf16)
    xs = x.reshape(NCORES, NPER, C, HW).astype(bf16)
    return [
        {
            "x": np.ascontiguousarray(xs[i]),
            "p1": p1h, "wt": wth, "p4": p4h, "p5": p5h,
        }
        for i in range(NCORES)
    ]


def _get_exec():
    """Compile the 8-core PJRT executable once; reuse across kernel() calls."""
    if "exec" in _CACHE:
        return _CACHE["exec"]
    import jax
    from jax.sharding import Mesh, NamedSharding, PartitionSpec
    from jax.experimental.shard_map import shard_map
    import concourse.mybir as mybir
    from concourse.bass2jax import (
        _bass_exec_p,
        install_neuronx_cc_hook,
        partition_id_tensor,
    )

    nc = _get_nc()
    install_neuronx_cc_hook()

    partition_name = (
        nc.partition_id_tensor.name if nc.partition_id_tensor else None
    )
    in_names, out_names, out_avals, zero_shapes = [], [], [], []
    for alloc in nc.m.functions[0].allocations:
        if not isinstance(alloc, mybir.MemoryLocationSet):
            continue
        name = alloc.memorylocations[0].name
        if alloc.kind == "ExternalInput":
            if name != partition_name:
                in_names.append(name)
        elif alloc.kind == "ExternalOutput":
            shape = tuple(alloc.tensor_shape)
            dtype = mybir.dt.np(alloc.dtype)
            out_avals.append(jax.core.ShapedArray(shape, dtype))
            out_names.append(name)
            zero_shapes.append((shape, dtype))
    n_params = len(in_names)
    all_in_names = list(in_names) + list(out_names)
    if partition_name is not None:
        all_in_names.append(partition_name)

    def _body(*args):
        operands = list(args)
        if partition_name is not None:
            operands.append(partition_id_tensor())
        outs = _bass_exec_p.bind(
            *operands,
            out_avals=tuple(out_avals),
            in_names=tuple(all_in_names),
            out_names=tuple(out_names),
            lowering_input_output_aliases=(),
            sim_require_finite=True,
            sim_require_nnan=True,
            nc=nc,
        )
        return tuple(outs)

    devices = jax.devices()[:NCORES]
    mesh = Mesh(np.asarray(devices), ("core",))
    nspecs = n_params + len(out_names)
    fn = jax.jit(
        shard_map(
            _body,
            mesh=mesh,
            in_specs=(PartitionSpec("core"),) * nspecs,
            out_specs=(PartitionSpec("core"),) * len(out_names),
            check_rep=False,
        ),
        keep_unused=True,
    )
    sharding = NamedSharding(mesh, PartitionSpec("core"))
    _CACHE["exec"] = (fn, in_names, out_names, out_avals, zero_shapes, sharding)
    return _CACHE["exec"]


def _run_fast(in_maps):
    """Cached sharded-PJRT executable: no retrace/recompile on repeat calls."""
    import jax

    fn, in_names, out_names, out_avals, zero_shapes, sharding = _get_exec()
    concat_in = [
        np.concatenate([m[nm] for m in in_maps], axis=0) for nm in in_names
    ]
    concat_zeros = [
        np.zeros((NCORES * s[0], *s[1:]), d) for (s, d) in zero_shapes
    ]
    dargs = [jax.device_put(a, sharding) for a in concat_in + concat_zeros]
    out_arrs = fn(*dargs)
    return np.asarray(out_arrs[0], dtype=np.float32)  # (N, 2, 128, HW)


def _run_spmd(in_maps):
    """Portable path via bass_utils (works on native-NRT machines too)."""
    from concourse.bass_utils import run_bass_kernel_spmd

    res = run_bass_kernel_spmd(
        _get_nc(), in_maps, core_ids=list(range(NCORES))
    )
    return np.concatenate(
        [np.asarray(res.results[i]["out"], dtype=np.float32)
         for i in range(NCORES)],
        axis=0,
    )


def kernel(**inputs):
    in_maps = host_prep(inputs)
    if _CACHE.get("fast_ok", True):
        try:
            out = _run_fast(in_maps)
            return out.reshape(N, C, H, W)
        except Exception:
            _CACHE["fast_ok"] = False
    out = _run_spmd(in_maps)
    return out.reshape(N, C, H, W)
